# revision 1
# baseline (speedup 1.0000x reference)
"""Paged-KV-cache causal GQA attention on 8 TRN2 NeuronCores.

Problem shape (hardcoded): B=8 seqs x S=1024 tokens, H=32 q-heads,
KVH=8 kv-heads (GQA group 4), D=128, block_size=256, 40 cache blocks.

Sharding: data parallel, one sequence per core. Host does the
store_kvcache scatter + block-table gather (layout work) and per-core
layout prep (head-major transposes + bf16 cast, scale folded into q);
each core runs causal flash attention for its sequence over all 32
heads.

Device algorithm per (head, q-chunk of 512), two heads interleaved:
  phase 1 (per k-tile pair): scores^T[k,q] = K^T.T @ Q^T  (PE, bf16)
           P = exp(scores)  (ACT, psum->sbuf bf16, no max subtraction:
           scores ~N(0,1)); diagonal block masked into a separate tile
           (DVE) so P keeps a single writer
  phase 2 (per q-tile): O[q,0:128]+rowsum[q] = P.T @ [V|1] accumulated
           over its k tiles back-to-back (PE), then out = O * (1/rowsum)
           (DVE) and DMA out.
Score psum tiles triple-buffered (3x2 banks) + po double-buffered
(2x1 bank) = 8 psum banks.
"""

import sys

import numpy as np
import ml_dtypes

sys.path.insert(0, "/opt/trn_rl_repo")

import concourse.bass as bass  # noqa: E402
import concourse.mybir as mybir  # noqa: E402
import concourse.tile as tile  # noqa: E402
from concourse import bacc  # noqa: E402
from concourse.bass_utils import run_bass_kernel_spmd  # noqa: E402

B, S = 8, 1024
H, KVH, D = 32, 8, 128
G = H // KVH
NT = S // 128  # 8 k/q tiles of 128 per sequence
VW = 132  # v tile row: 128 v cols + ones col + pad
SCALE = 1.0 / float(np.sqrt(D))
BF = mybir.dt.bfloat16
F32 = mybir.dt.float32
_NC = None


def _kts(qc):
    """k-tiles of q-chunk qc as (kt, q_off, width)."""
    return [
        (kt, max(0, kt - qc * 4), 512 - max(0, kt - qc * 4) * 128)
        for kt in range(qc * 4 + 4)
    ]


def _build_nc():
    nc = bacc.Bacc("TRN2", target_bir_lowering=False, debug=False, num_devices=8)
    qT = nc.dram_tensor("qT", [H, D, S], BF, kind="ExternalInput").ap()
    kT = nc.dram_tensor("kT", [KVH, D, S], BF, kind="ExternalInput").ap()
    v1 = nc.dram_tensor("v1", [KVH, NT, 128, VW], BF, kind="ExternalInput").ap()
    out = nc.dram_tensor("out", [H, S, D], F32, kind="ExternalOutput").ap()
    mask_np = np.triu(np.ones((128, 128), dtype=ml_dtypes.bfloat16))
    mask_dram = nc.inline_tensor(mask_np, "tri_mask").ap()

    with tile.TileContext(nc) as tc:
        with (
            tc.tile_pool(name="singles", bufs=1) as singles,
            tc.tile_pool(name="qpool", bufs=6) as qpool,
            tc.tile_pool(name="ppool", bufs=17) as ppool,
            tc.tile_pool(name="dpool", bufs=22) as dpool,
            tc.tile_pool(name="opool", bufs=8) as opool,
            tc.tile_pool(name="rpool", bufs=10) as rpool,
            tc.tile_pool(name="pspool", bufs=3, space="PSUM") as pspool,
            tc.tile_pool(name="popool", bufs=2, space="PSUM") as popool,
        ):
            mask_sb = singles.tile([128, 128], BF)
            kv_sb = []
            for kvh in range(KVH):
                k_t = singles.tile([128, S], BF, name=f"kT_sb{kvh}", tag=f"kT{kvh}")
                v_t = singles.tile(
                    [128, NT * VW], BF, name=f"v1_sb{kvh}", tag=f"v1{kvh}"
                )
                kv_sb.append((k_t, v_t))

            def load_kv(kvh):
                nc.sync.dma_start(out=kv_sb[kvh][0], in_=kT[kvh])
                nc.sync.dma_start(
                    out=kv_sb[kvh][1].rearrange("p (t c) -> p t c", t=NT),
                    in_=v1[kvh].rearrange("t p c -> p t c"),
                )

            q_tiles = {}

            def load_q(h):
                if h < H and h not in q_tiles:
                    q_tiles[h] = qpool.tile([128, S], BF, tag="q", name=f"q_sb{h}")
                    nc.sync.dma_start(out=q_tiles[h], in_=qT[h])

            # fast start: first pair's k-tiles + first q chunk land first so
            # head 0's first scores/exp start as early as possible
            q_tiles[0] = qpool.tile([128, S], BF, tag="q", name="q_sb0")
            nc.sync.dma_start(out=kv_sb[0][0][:, 0:256], in_=kT[0][:, 0:256])
            nc.sync.dma_start(out=q_tiles[0][:, 0:512], in_=qT[0][:, 0:512])
            q_tiles[1] = qpool.tile([128, S], BF, tag="q", name="q_sb1")
            nc.sync.dma_start(out=q_tiles[1][:, 0:512], in_=qT[1][:, 0:512])
            nc.sync.dma_start(out=kv_sb[0][0][:, 256:], in_=kT[0][:, 256:])
            nc.sync.dma_start(out=mask_sb, in_=mask_dram)
            nc.sync.dma_start(
                out=kv_sb[0][1].rearrange("p (t c) -> p t c", t=NT)[:, 0:2, :],
                in_=v1[0].rearrange("t p c -> p t c")[:, 0:2, :],
            )
            nc.sync.dma_start(out=q_tiles[0][:, 512:], in_=qT[0][:, 512:])
            nc.sync.dma_start(out=q_tiles[1][:, 512:], in_=qT[1][:, 512:])
            nc.sync.dma_start(
                out=kv_sb[0][1].rearrange("p (t c) -> p t c", t=NT)[:, 2:, :],
                in_=v1[0].rearrange("t p c -> p t c")[:, 2:, :],
            )
            for h in range(2, 4):
                load_q(h)
            load_kv(1)

            for h0 in range(0, H, 2):
                hs = (h0, h0 + 1)
                kvh = h0 // G
                kT_sb, v1_sb = kv_sb[kvh]
                load_q(h0 + 2)
                load_q(h0 + 3)
                if h0 % G == 0 and kvh + 2 < KVH:
                    load_kv(kvh + 2)
                pairs_all = []
                for qc in range(2):
                    kts = _kts(qc)
                    pairs_all += [
                        (qc, kts[i : i + 2]) for i in range(0, len(kts), 2)
                    ]
                p_loc = {h: {} for h in hs}
                d_sb = {h: {} for h in hs}
                osb_c = {
                    h: {
                        qc: opool.tile(
                            [128, 512], F32, tag="o", name=f"o_{h}_{qc}"
                        )
                        for qc in range(2)
                    }
                    for h in hs
                }
                osb_n = {h: {0: 0, 1: 0} for h in hs}

                po2 = {}

                def pv_run(h, qc, qt, start_kt=0):
                    # accumulate P.T @ [V|1] over qt's k tiles back-to-back;
                    # two q-tiles share one psum bank (single start=True per
                    # bank), reciprocal batched over both rowsums
                    if qt % 2 == 0 and start_kt == 0:
                        po2[h] = popool.tile(
                            [128, 258], F32, tag="po", name=f"po_{h}_{qt}"
                        )
                    po = po2[h]
                    base = (qt % 2) * 129
                    for kt in range(start_kt, qt + 1):
                        if kt == qt:
                            lhsT = d_sb[h][(qc, kt)]
                        else:
                            t, pb = p_loc[h][(qc, kt)]
                            q_off = max(0, kt - qc * 4)
                            j = qt - qc * 4
                            lhsT = t[
                                :,
                                pb + (j - q_off) * 128 : pb
                                + (j - q_off) * 128
                                + 128,
                            ]
                        nc.tensor.matmul(
                            po[:, base : base + 129],
                            lhsT=lhsT,
                            rhs=v1_sb[:, kt * VW : kt * VW + 129],
                            start=(kt == 0 and qt % 2 == 0 and start_kt == 0),
                            stop=(kt == qt),
                            skip_group_check=True,
                        )
                    if qt % 2 == 0:
                        return
                    recip = rpool.tile([128, 2], F32, tag="r", name=f"r_{h}_{qt}")
                    nc.vector.reciprocal(
                        recip, po.rearrange("p (a b) -> p a b", a=2)[:, :, 128]
                    )
                    for q2, r2 in ((qt - 1, 0), (qt, 1)):
                        j = q2 - qc * 4
                        b2 = (q2 % 2) * 129
                        if h0 == H - 2 and qc == 1:
                            # tail: per-q-tile store on the now-idle Sync ring
                            osb = opool.tile(
                                [128, 128], F32, tag="o", name=f"ot_{h}_{q2}"
                            )
                            nc.vector.tensor_scalar_mul(
                                osb, po[:, b2 : b2 + 128], recip[:, r2 : r2 + 1]
                            )
                            nc.sync.dma_start(
                                out=out[h, q2 * 128 : (q2 + 1) * 128, :], in_=osb
                            )
                            continue
                        nc.vector.tensor_scalar_mul(
                            osb_c[h][qc][:, j * 128 : j * 128 + 128],
                            po[:, b2 : b2 + 128],
                            recip[:, r2 : r2 + 1],
                        )
                        osb_n[h][qc] += 1
                    if osb_n[h][qc] == 4:
                        # one 256KB store per (head, chunk) from the GpSimd
                        # sequencer; keeps the Sync HWDGE ring free for loads
                        nc.gpsimd.dma_start(
                            out=out[h, qc * 512 : (qc + 1) * 512, :].rearrange(
                                "(t p) d -> p t d", p=128
                            ),
                            in_=osb_c[h][qc].rearrange("p (t d) -> p t d", t=4),
                        )

                pending = []
                for qc, pair in pairs_all:
                    # tight-pack the pair: second k-tile shares the first's
                    # psum bank when both fit (its matmul then uses
                    # start=False — the first matmul's bank-wide has_written
                    # clear lets it overwrite); no garbage columns in the exp
                    offs = [0]
                    if len(pair) == 2:
                        w0, w1 = pair[0][2], pair[1][2]
                        offs.append(w0 if w0 + w1 <= 512 else 512)
                    tw = offs[-1] + pair[-1][2]
                    # both heads' scores + exp for this pair
                    for h in hs:
                        ps = pspool.tile(
                            [128, tw], F32, tag="ps",
                            name=f"ps_{h}_{qc}_{pair[0][0]}",
                        )
                        for pi, (kt, q_off, w) in enumerate(pair):
                            nc.tensor.matmul(
                                ps[:, offs[pi] : offs[pi] + w],
                                lhsT=kT_sb[:, kt * 128 : kt * 128 + 128],
                                rhs=q_tiles[h][
                                    :, qc * 512 + q_off * 128 : qc * 512 + 512
                                ],
                                start=(offs[pi] % 512 == 0),
                                stop=True,
                                skip_group_check=True,
                            )
                        p_sb = ppool.tile(
                            [128, tw], BF, tag="p",
                            name=f"p_{h}_{qc}_{pair[0][0]}",
                        )
                        # P = exp(scores); scale pre-folded into q on host
                        nc.scalar.activation(
                            p_sb, ps, mybir.ActivationFunctionType.Exp
                        )
                        for pi, (kt, q_off, w) in enumerate(pair):
                            p_loc[h][(qc, kt)] = (p_sb, offs[pi])
                            if kt >= qc * 4:  # diagonal: upper-tri mask
                                dt_ = dpool.tile(
                                    [128, 128], BF, tag="d",
                                    name=f"d_{h}_{qc}_{kt}",
                                )
                                nc.vector.tensor_mul(
                                    dt_, p_sb[:, offs[pi] : offs[pi] + 128],
                                    mask_sb,
                                )
                                d_sb[h][(qc, kt)] = dt_
                    # last unit, pair F: pre-accumulate qt6/qt7 over kt0..5
                    # now so only the diagonal matmuls trail the final exp
                    # (shorter kernel tail); these run during F's exps
                    if h0 == H - 2 and qc == 1 and pair[0][0] == 6:
                        for h2 in hs:
                            po2[h2] = popool.tile(
                                [128, 258], F32, tag="po", name=f"po_{h2}_6"
                            )
                            for qt in (6, 7):
                                b2 = (qt % 2) * 129
                                j = qt - 4
                                for kt in range(6):
                                    t, pb = p_loc[h2][(1, kt)]
                                    q_off = max(0, kt - 4)
                                    nc.tensor.matmul(
                                        po2[h2][:, b2 : b2 + 129],
                                        lhsT=t[
                                            :,
                                            pb + (j - q_off) * 128 : pb
                                            + (j - q_off) * 128
                                            + 128,
                                        ],
                                        rhs=v1_sb[:, kt * VW : kt * VW + 129],
                                        start=(kt == 0 and qt == 6),
                                        stop=False,
                                        skip_group_check=True,
                                    )
                    # emit PV runs one pair late so the next pair's QK + exp
                    # stay ahead of the PV burst on the PE stream (eager on the
                    # last unit to shorten the kernel tail)
                    for args in pending:
                        pv_run(*args)
                    pending = [
                        (h, qc, kt)
                        for h in hs
                        for kt, q_off, w in pair
                        if kt >= qc * 4
                    ]
                    if h0 == H - 2:
                        for args in pending:
                            h3, qc3, qt3 = args
                            pv_run(h3, qc3, qt3, start_kt=6 if qt3 >= 6 else 0)
                        pending = []
                for args in pending:
                    pv_run(*args)

    nc.compile()
    return nc


def _get_nc():
    global _NC
    if _NC is None:
        _NC = _build_nc()
    return _NC


def make_in_maps(q, k, v, k_cache, v_cache, slot_mapping, block_tables):
    nb, bs, kvh, d = k_cache.shape
    # store_kvcache scatter (mirrors reference semantics on host)
    kc = k_cache.reshape(nb * bs, kvh, d).copy()
    vc = v_cache.reshape(nb * bs, kvh, d).copy()
    kc[slot_mapping] = k
    vc[slot_mapping] = v
    b, mb = block_tables.shape
    s = q.shape[0] // b
    pos = np.arange(s)
    slot_grid = block_tables[:, pos // bs] * bs + (pos % bs)  # [B, S]
    kf = kc[slot_grid]  # [B, S, KVH, D]
    vf = vc[slot_grid]
    qb = q.reshape(b, s, H, D)

    bf16 = ml_dtypes.bfloat16
    in_maps = []
    for i in range(b):
        qTi = np.ascontiguousarray(
            qb[i].transpose(1, 2, 0) * np.float32(SCALE)
        ).astype(bf16)
        kTi = np.ascontiguousarray(kf[i].transpose(1, 2, 0)).astype(bf16)
        vh = vf[i].transpose(1, 0, 2).reshape(KVH, NT, 128, D)
        v1i = np.zeros((KVH, NT, 128, VW), dtype=bf16)
        v1i[..., :D] = vh.astype(bf16)
        v1i[..., D] = 1.0
        in_maps.append({"qT": qTi, "kT": kTi, "v1": v1i})
    return in_maps


def kernel(q, k, v, k_cache, v_cache, slot_mapping, block_tables):
    # accept jax or numpy inputs
    q = np.asarray(q)
    k = np.asarray(k)
    v = np.asarray(v)
    k_cache = np.asarray(k_cache)
    v_cache = np.asarray(v_cache)
    slot_mapping = np.asarray(slot_mapping)
    block_tables = np.asarray(block_tables)
    out_dtype = q.dtype
    in_maps = make_in_maps(q, k, v, k_cache, v_cache, slot_mapping, block_tables)
    nc = _get_nc()
    res = run_bass_kernel_spmd(nc, in_maps, core_ids=list(range(8)))
    outs = [res.results[i]["out"].transpose(1, 0, 2) for i in range(B)]  # [S, H, D]
    return np.concatenate(outs, axis=0).astype(out_dtype, copy=False)



# revision 2
# speedup vs baseline: 1.2031x; 1.2031x over previous
"""Paged-KV-cache causal GQA attention on 8 TRN2 NeuronCores.

Problem shape (hardcoded): B=8 seqs x S=1024 tokens, H=32 q-heads,
KVH=8 kv-heads (GQA group 4), D=128, block_size=256, 40 cache blocks.

Sharding: data parallel, one sequence per core. Host does the
store_kvcache scatter + block-table gather (layout work) and per-core
layout prep (head-major transposes + bf16 cast, scale folded into q);
each core runs causal flash attention for its sequence over all 32
heads.

Device algorithm per (head, q-chunk of 512), two heads interleaved:
  phase 1 (per k-tile pair): scores^T[k,q] = K^T.T @ Q^T  (PE, bf16)
           P = exp(scores)  (ACT, psum->sbuf bf16, no max subtraction:
           scores ~N(0,1)); diagonal block masked into a separate tile
           (DVE) so P keeps a single writer
  phase 2 (per q-tile): O[q,0:128]+rowsum[q] = P.T @ [V|1] accumulated
           over its k tiles back-to-back (PE), then out = O * (1/rowsum)
           (DVE) and DMA out.
Score psum tiles triple-buffered (3x2 banks) + po double-buffered
(2x1 bank) = 8 psum banks.
"""

import sys

import numpy as np
import ml_dtypes

sys.path.insert(0, "/opt/trn_rl_repo")

import concourse.bass as bass  # noqa: E402
import concourse.mybir as mybir  # noqa: E402
import concourse.tile as tile  # noqa: E402
from concourse import bacc  # noqa: E402
from concourse.bass_utils import run_bass_kernel_spmd  # noqa: E402

B, S = 8, 1024
H, KVH, D = 32, 8, 128
G = H // KVH
NT = S // 128  # 8 k/q tiles of 128 per sequence
VW = 132  # v tile row: 128 v cols + ones col + pad
SCALE = 1.0 / float(np.sqrt(D))
BF = mybir.dt.bfloat16
F32 = mybir.dt.float32
_NC = None


def _kts(qc):
    """k-tiles of q-chunk qc as (kt, q_off, width)."""
    return [
        (kt, max(0, kt - qc * 4), 512 - max(0, kt - qc * 4) * 128)
        for kt in range(qc * 4 + 4)
    ]


def _build_nc():
    nc = bacc.Bacc("TRN2", target_bir_lowering=False, debug=False, num_devices=8)
    qT = nc.dram_tensor("qT", [H, D, S], BF, kind="ExternalInput").ap()
    kT = nc.dram_tensor("kT", [KVH, D, S], BF, kind="ExternalInput").ap()
    v1 = nc.dram_tensor("v1", [KVH, NT, 128, VW], BF, kind="ExternalInput").ap()
    out = nc.dram_tensor("out", [H, S, D], F32, kind="ExternalOutput").ap()
    mask_np = np.triu(np.ones((128, 128), dtype=ml_dtypes.bfloat16))
    mask_dram = nc.inline_tensor(mask_np, "tri_mask").ap()

    with tile.TileContext(nc) as tc:
        with (
            tc.tile_pool(name="singles", bufs=1) as singles,
            tc.tile_pool(name="qpool", bufs=6) as qpool,
            tc.tile_pool(name="ppool", bufs=17) as ppool,
            tc.tile_pool(name="dpool", bufs=22) as dpool,
            tc.tile_pool(name="opool", bufs=8) as opool,
            tc.tile_pool(name="rpool", bufs=10) as rpool,
            tc.tile_pool(name="pspool", bufs=3, space="PSUM") as pspool,
            tc.tile_pool(name="popool", bufs=2, space="PSUM") as popool,
        ):
            mask_sb = singles.tile([128, 128], BF)
            kv_sb = []
            for kvh in range(KVH):
                k_t = singles.tile([128, S], BF, name=f"kT_sb{kvh}", tag=f"kT{kvh}")
                v_t = singles.tile(
                    [128, NT * VW], BF, name=f"v1_sb{kvh}", tag=f"v1{kvh}"
                )
                kv_sb.append((k_t, v_t))

            def load_kv(kvh):
                nc.sync.dma_start(out=kv_sb[kvh][0], in_=kT[kvh])
                nc.sync.dma_start(
                    out=kv_sb[kvh][1].rearrange("p (t c) -> p t c", t=NT),
                    in_=v1[kvh].rearrange("t p c -> p t c"),
                )

            q_tiles = {}

            def load_q(h):
                if h < H and h not in q_tiles:
                    q_tiles[h] = qpool.tile([128, S], BF, tag="q", name=f"q_sb{h}")
                    nc.sync.dma_start(out=q_tiles[h], in_=qT[h])

            # fast start: first pair's k-tiles + first q chunk land first so
            # head 0's first scores/exp start as early as possible
            q_tiles[0] = qpool.tile([128, S], BF, tag="q", name="q_sb0")
            nc.sync.dma_start(out=kv_sb[0][0][:, 0:256], in_=kT[0][:, 0:256])
            nc.sync.dma_start(out=q_tiles[0][:, 0:512], in_=qT[0][:, 0:512])
            q_tiles[1] = qpool.tile([128, S], BF, tag="q", name="q_sb1")
            nc.sync.dma_start(out=q_tiles[1][:, 0:512], in_=qT[1][:, 0:512])
            nc.sync.dma_start(out=kv_sb[0][0][:, 256:], in_=kT[0][:, 256:])
            nc.sync.dma_start(out=mask_sb, in_=mask_dram)
            nc.sync.dma_start(
                out=kv_sb[0][1].rearrange("p (t c) -> p t c", t=NT)[:, 0:2, :],
                in_=v1[0].rearrange("t p c -> p t c")[:, 0:2, :],
            )
            nc.sync.dma_start(out=q_tiles[0][:, 512:], in_=qT[0][:, 512:])
            nc.sync.dma_start(out=q_tiles[1][:, 512:], in_=qT[1][:, 512:])
            nc.sync.dma_start(
                out=kv_sb[0][1].rearrange("p (t c) -> p t c", t=NT)[:, 2:, :],
                in_=v1[0].rearrange("t p c -> p t c")[:, 2:, :],
            )
            for h in range(2, 4):
                load_q(h)
            load_kv(1)

            for h0 in range(0, H, 2):
                hs = (h0, h0 + 1)
                kvh = h0 // G
                kT_sb, v1_sb = kv_sb[kvh]
                load_q(h0 + 2)
                load_q(h0 + 3)
                if h0 % G == 0 and kvh + 2 < KVH:
                    load_kv(kvh + 2)
                pairs_all = []
                for qc in range(2):
                    kts = _kts(qc)
                    pairs_all += [
                        (qc, kts[i : i + 2]) for i in range(0, len(kts), 2)
                    ]
                p_loc = {h: {} for h in hs}
                d_sb = {h: {} for h in hs}
                osb_c = {
                    h: {
                        qc: opool.tile(
                            [128, 512], F32, tag="o", name=f"o_{h}_{qc}"
                        )
                        for qc in range(2)
                    }
                    for h in hs
                }
                osb_n = {h: {0: 0, 1: 0} for h in hs}

                po2 = {}

                def pv_run(h, qc, qt, start_kt=0):
                    # accumulate P.T @ [V|1] over qt's k tiles back-to-back;
                    # two q-tiles share one psum bank (single start=True per
                    # bank), reciprocal batched over both rowsums
                    if qt % 2 == 0 and start_kt == 0:
                        po2[h] = popool.tile(
                            [128, 258], F32, tag="po", name=f"po_{h}_{qt}"
                        )
                    po = po2[h]
                    base = (qt % 2) * 129
                    for kt in range(start_kt, qt + 1):
                        if kt == qt:
                            lhsT = d_sb[h][(qc, kt)]
                        else:
                            t, pb = p_loc[h][(qc, kt)]
                            q_off = max(0, kt - qc * 4)
                            j = qt - qc * 4
                            lhsT = t[
                                :,
                                pb + (j - q_off) * 128 : pb
                                + (j - q_off) * 128
                                + 128,
                            ]
                        nc.tensor.matmul(
                            po[:, base : base + 129],
                            lhsT=lhsT,
                            rhs=v1_sb[:, kt * VW : kt * VW + 129],
                            start=(kt == 0 and qt % 2 == 0 and start_kt == 0),
                            stop=(kt == qt),
                            skip_group_check=True,
                        )
                    if qt % 2 == 0:
                        return
                    recip = rpool.tile([128, 2], F32, tag="r", name=f"r_{h}_{qt}")
                    nc.vector.reciprocal_approx_fast(
                        recip, po.rearrange("p (a b) -> p a b", a=2)[:, :, 128]
                    )
                    for q2, r2 in ((qt - 1, 0), (qt, 1)):
                        j = q2 - qc * 4
                        b2 = (q2 % 2) * 129
                        if h0 == H - 2 and qc == 1:
                            # tail: per-q-tile store on the now-idle Sync ring
                            osb = opool.tile(
                                [128, 128], F32, tag="o", name=f"ot_{h}_{q2}"
                            )
                            nc.vector.tensor_scalar_mul(
                                osb, po[:, b2 : b2 + 128], recip[:, r2 : r2 + 1]
                            )
                            nc.sync.dma_start(
                                out=out[h, q2 * 128 : (q2 + 1) * 128, :], in_=osb
                            )
                            continue
                        nc.vector.tensor_scalar_mul(
                            osb_c[h][qc][:, j * 128 : j * 128 + 128],
                            po[:, b2 : b2 + 128],
                            recip[:, r2 : r2 + 1],
                        )
                        osb_n[h][qc] += 1
                    if osb_n[h][qc] == 4:
                        # one 256KB store per (head, chunk) from the GpSimd
                        # sequencer; keeps the Sync HWDGE ring free for loads
                        nc.gpsimd.dma_start(
                            out=out[h, qc * 512 : (qc + 1) * 512, :].rearrange(
                                "(t p) d -> p t d", p=128
                            ),
                            in_=osb_c[h][qc].rearrange("p (t d) -> p t d", t=4),
                        )

                pending = []
                for qc, pair in pairs_all:
                    # tight-pack the pair: second k-tile shares the first's
                    # psum bank when both fit (its matmul then uses
                    # start=False — the first matmul's bank-wide has_written
                    # clear lets it overwrite); no garbage columns in the exp
                    offs = [0]
                    if len(pair) == 2:
                        w0, w1 = pair[0][2], pair[1][2]
                        offs.append(w0 if w0 + w1 <= 512 else 512)
                    tw = offs[-1] + pair[-1][2]
                    # both heads' scores + exp for this pair
                    for h in hs:
                        ps = pspool.tile(
                            [128, tw], F32, tag="ps",
                            name=f"ps_{h}_{qc}_{pair[0][0]}",
                        )
                        for pi, (kt, q_off, w) in enumerate(pair):
                            nc.tensor.matmul(
                                ps[:, offs[pi] : offs[pi] + w],
                                lhsT=kT_sb[:, kt * 128 : kt * 128 + 128],
                                rhs=q_tiles[h][
                                    :, qc * 512 + q_off * 128 : qc * 512 + 512
                                ],
                                start=(offs[pi] % 512 == 0),
                                stop=True,
                                skip_group_check=True,
                            )
                        p_sb = ppool.tile(
                            [128, tw], BF, tag="p",
                            name=f"p_{h}_{qc}_{pair[0][0]}",
                        )
                        # P = exp(scores); scale pre-folded into q on host
                        nc.scalar.activation(
                            p_sb, ps, mybir.ActivationFunctionType.Exp
                        )
                        for pi, (kt, q_off, w) in enumerate(pair):
                            p_loc[h][(qc, kt)] = (p_sb, offs[pi])
                            if kt >= qc * 4:  # diagonal: upper-tri mask
                                dt_ = dpool.tile(
                                    [128, 128], BF, tag="d",
                                    name=f"d_{h}_{qc}_{kt}",
                                )
                                nc.vector.tensor_mul(
                                    dt_, p_sb[:, offs[pi] : offs[pi] + 128],
                                    mask_sb,
                                )
                                d_sb[h][(qc, kt)] = dt_
                    # last unit, pair F: pre-accumulate qt6/qt7 over kt0..5
                    # now so only the diagonal matmuls trail the final exp
                    # (shorter kernel tail); these run during F's exps
                    if h0 == H - 2 and qc == 1 and pair[0][0] == 6:
                        for h2 in hs:
                            po2[h2] = popool.tile(
                                [128, 258], F32, tag="po", name=f"po_{h2}_6"
                            )
                            for qt in (6, 7):
                                b2 = (qt % 2) * 129
                                j = qt - 4
                                for kt in range(6):
                                    t, pb = p_loc[h2][(1, kt)]
                                    q_off = max(0, kt - 4)
                                    nc.tensor.matmul(
                                        po2[h2][:, b2 : b2 + 129],
                                        lhsT=t[
                                            :,
                                            pb + (j - q_off) * 128 : pb
                                            + (j - q_off) * 128
                                            + 128,
                                        ],
                                        rhs=v1_sb[:, kt * VW : kt * VW + 129],
                                        start=(kt == 0 and qt == 6),
                                        stop=False,
                                        skip_group_check=True,
                                    )
                    # emit PV runs one pair late so the next pair's QK + exp
                    # stay ahead of the PV burst on the PE stream (eager on the
                    # last unit to shorten the kernel tail)
                    for args in pending:
                        pv_run(*args)
                    pending = [
                        (h, qc, kt)
                        for h in hs
                        for kt, q_off, w in pair
                        if kt >= qc * 4
                    ]
                    if h0 == H - 2:
                        for args in pending:
                            h3, qc3, qt3 = args
                            pv_run(h3, qc3, qt3, start_kt=6 if qt3 >= 6 else 0)
                        pending = []
                for args in pending:
                    pv_run(*args)

    nc.compile()
    return nc


def _get_nc():
    global _NC
    if _NC is None:
        _NC = _build_nc()
    return _NC


def make_in_maps(q, k, v, k_cache, v_cache, slot_mapping, block_tables):
    nb, bs, kvh, d = k_cache.shape
    # store_kvcache scatter (mirrors reference semantics on host)
    kc = k_cache.reshape(nb * bs, kvh, d).copy()
    vc = v_cache.reshape(nb * bs, kvh, d).copy()
    kc[slot_mapping] = k
    vc[slot_mapping] = v
    b, mb = block_tables.shape
    s = q.shape[0] // b
    pos = np.arange(s)
    slot_grid = block_tables[:, pos // bs] * bs + (pos % bs)  # [B, S]
    kf = kc[slot_grid]  # [B, S, KVH, D]
    vf = vc[slot_grid]
    qb = q.reshape(b, s, H, D)

    bf16 = ml_dtypes.bfloat16
    in_maps = []
    for i in range(b):
        qTi = np.ascontiguousarray(
            qb[i].transpose(1, 2, 0) * np.float32(SCALE)
        ).astype(bf16)
        kTi = np.ascontiguousarray(kf[i].transpose(1, 2, 0)).astype(bf16)
        vh = vf[i].transpose(1, 0, 2).reshape(KVH, NT, 128, D)
        v1i = np.zeros((KVH, NT, 128, VW), dtype=bf16)
        v1i[..., :D] = vh.astype(bf16)
        v1i[..., D] = 1.0
        in_maps.append({"qT": qTi, "kT": kTi, "v1": v1i})
    return in_maps


def kernel(q, k, v, k_cache, v_cache, slot_mapping, block_tables):
    # accept jax or numpy inputs
    q = np.asarray(q)
    k = np.asarray(k)
    v = np.asarray(v)
    k_cache = np.asarray(k_cache)
    v_cache = np.asarray(v_cache)
    slot_mapping = np.asarray(slot_mapping)
    block_tables = np.asarray(block_tables)
    out_dtype = q.dtype
    in_maps = make_in_maps(q, k, v, k_cache, v_cache, slot_mapping, block_tables)
    nc = _get_nc()
    res = run_bass_kernel_spmd(nc, in_maps, core_ids=list(range(8)))
    outs = [res.results[i]["out"].transpose(1, 0, 2) for i in range(B)]  # [S, H, D]
    return np.concatenate(outs, axis=0).astype(out_dtype, copy=False)

